# revision 4
# baseline (speedup 1.0000x reference)
"""Trainium2 Bass kernel for nn_BinarizeLayer.

out[b, f] = (medians[f] > 0) AND (inputs[b, f] >= medians[f])

Host preprocessing folds the two conditions into one comparison AND
prunes dead features: for the ~50% of features with medians[f] <= 0 the
output is False regardless of the input, so those input columns are
never shipped to (or read by) the device at all. The host gathers the
Fs = #(medians > 0) "live" columns into a compact [B, FP] array
(FP = Fs rounded up to a multiple of 1024), the device computes
out = x_gathered >= medians_gathered, and the host scatters the packed
result back into the full [B, 8192] output (False elsewhere). This
halves HBM read traffic, which is the roofline for this kernel.

Data-parallel over 8 NeuronCores, processed as chunks of 64 consecutive
rows (~1 MiB, fully contiguous in DRAM). The load's access pattern fans
each chunk onto 128 partitions: partition p holds half-row
(row 64i + p//2, cols (p%2)*CQ ..), keeping HBM reads sequential with
8 KiB descriptor lines. Each chunk's load is SPLIT across two DMA
rings (sync: partitions 0-63, gpsimd: partitions 64-127): a single
queue serializes descriptor execution at ~280 GB/s; two queues stream
concurrently toward the per-core HBM ceiling.

Per chunk: DVE compares against a median tile host-prepared in the same
per-partition layout, emitting 0/1 bf16 bits; the tensor engine
bit-packs 8 partitions per byte with one constant [128,16] matmul
weight (2^(p%8) block-diagonal), accumulating exact small integers in
PSUM. PSUM is evacuated to SBUF with an f32->u8 cast, ALTERNATING
between the scalar and vector engines (a scalar activation instruction
has ~1.4us fixed overhead; 32 of them would approach the kernel's
whole budget). Each core stores ~1 MiB instead of ~8 MiB; the host
unpacks bits and inverts the partition bijection with pure reshapes.

Tail: every core's last chunk is loaded whole but compared/packed in
<=512-column slabs so the post-load drain is short and overlaps the
previous chunks' stores.
"""

import numpy as np

import concourse.bacc as bacc
import concourse.mybir as mybir
from concourse import tile
from concourse.bass_utils import run_bass_kernel_spmd

N_CORES = 8
B, F = 16384, 8192
P = 128  # SBUF partitions
G = P // 8  # packed bytes' groups per chunk (16)
MM_N = 512  # matmul free-dim chunk (one PSUM bank)
ROWS_PER_CORE = B // N_CORES  # 2048


def _geom(fs):
    """Padded width FP and chunk geometry for fs gathered columns.

    FP is a multiple of 1024 so matmul slab offsets stay PSUM-bank
    aligned. c = column segments per row (partition p holds segment
    p%c of row i*R + p//c); R = rows per chunk. c=2 keeps 8 KiB
    descriptor lines while cq = FP/c fits a [G, cq] f32 PSUM tile in
    4 banks (double-buffered); fall back to c=4 for FP > 4096.
    """
    fp = max(1024, -(-fs // 1024) * 1024)
    c = 2 if fp <= 4096 else 4
    r = P // c
    cq = fp // c
    return fp, c, r, cq


def _build(fp):
    """Build the SPMD program for FP = fp gathered (padded) columns."""
    fp, c, r, cq = _geom(fp)
    k = ROWS_PER_CORE // r  # chunks per core
    nc = bacc.Bacc(
        "TRN2",
        target_bir_lowering=False,
        debug=False,
        num_devices=N_CORES,
    )
    x = nc.declare_dram_parameter(
        "x", [ROWS_PER_CORE, fp], mybir.dt.float32, isOutput=False
    )
    med = nc.declare_dram_parameter(
        "med", [P // 4, cq], mybir.dt.float32, isOutput=False
    )
    pw = nc.declare_dram_parameter("pw", [P, G], mybir.dt.float32, isOutput=False)
    out = nc.declare_dram_parameter(
        "out", [k * G, cq], mybir.dt.uint8, isOutput=True
    )
    xv = x.rearrange("(i r) (c j) -> i (r c) j", r=r, c=c)

    # <=512-wide matmul slabs covering cq (last one may be partial).
    slabs = [(s, min(MM_N, cq - s)) for s in range(0, cq, MM_N)]
    H = P // 2

    with tile.TileContext(nc) as tc:
        with (
            tc.tile_pool(name="const", bufs=1) as cpool,
            tc.tile_pool(name="xp", bufs=12) as xpool,
            tc.tile_pool(name="bp", bufs=5) as bpool,
            tc.tile_pool(name="op", bufs=5) as opool,
            tc.tile_pool(name="ps", bufs=2, space="PSUM") as pspool,
        ):
            # Constants on the scalar ring; the load rings (sync,
            # gpsimd) are purely x-loads from instruction 0. The median
            # tile's layout is periodic every c partitions; a
            # 32-partition replica is loaded and doubled twice on the
            # DVE (partition bases must be 32-aligned).
            med_t = cpool.tile([P, cq], mybir.dt.float32)
            nc.scalar.dma_start(out=med_t[0 : P // 4, :], in_=med[:])
            nc.vector.tensor_copy(
                out=med_t[P // 4 : P // 2, :], in_=med_t[0 : P // 4, :]
            )
            nc.vector.tensor_copy(
                out=med_t[P // 2 : P, :], in_=med_t[0 : P // 2, :]
            )
            pw_f32 = cpool.tile([P, G], mybir.dt.float32)
            pw_t = cpool.tile([P, G], mybir.dt.bfloat16)
            nc.scalar.dma_start(out=pw_f32[:], in_=pw[:])
            nc.vector.tensor_copy(out=pw_t[:], in_=pw_f32[:])

            def load(i):
                xt = xpool.tile([P, cq], mybir.dt.float32, tag="x")
                nc.sync.dma_start(out=xt[:H, :], in_=xv[i][:H])
                nc.gpsimd.dma_start(out=xt[H:, :], in_=xv[i][H:])
                return xt

            def body(i):
                xt = load(i)
                bt = bpool.tile([P, cq], mybir.dt.bfloat16, tag="b")
                nc.vector.tensor_tensor(
                    bt[:], xt[:], med_t[:], mybir.AluOpType.is_ge
                )
                ps = pspool.tile([G, cq], mybir.dt.float32, tag="ps")
                for s, w in slabs:
                    nc.tensor.matmul(
                        ps[:, s : s + w],
                        pw_t[:],
                        bt[:, s : s + w],
                        start=True,
                        stop=True,
                    )
                pk = opool.tile([G, cq], mybir.dt.uint8, tag="o")
                if i % 2 == 0:
                    nc.scalar.copy(out=pk[:], in_=ps[:])
                else:
                    nc.vector.tensor_copy(out=pk[:], in_=ps[:])
                nc.scalar.dma_start(
                    out=out[i * G : (i + 1) * G, :], in_=pk[:]
                )

            def slab_tail(i):
                # Load whole, drain in <=512-col slabs on otherwise-idle
                # engines (DVE evac, tensor-ring stores).
                xt = load(i)
                ps = pspool.tile([G, cq], mybir.dt.float32, tag="ps")
                for s, w in slabs:
                    bt = bpool.tile([P, w], mybir.dt.bfloat16, tag="bt")
                    nc.vector.tensor_tensor(
                        bt[:],
                        xt[:, s : s + w],
                        med_t[:, s : s + w],
                        mybir.AluOpType.is_ge,
                    )
                    nc.tensor.matmul(
                        ps[:, s : s + w], pw_t[:], bt[:], start=True, stop=True
                    )
                    pk = opool.tile([G, w], mybir.dt.uint8, tag="ot")
                    nc.vector.tensor_copy(out=pk[:], in_=ps[:, s : s + w])
                    nc.scalar.dma_start(
                        out=out[i * G : (i + 1) * G, s : s + w], in_=pk[:]
                    )

            for i in range(k - 1):
                body(i)
            slab_tail(k - 1)
    nc.compile()
    return nc


def _pack_weights():
    pw = np.zeros((P, G), dtype=np.float32)
    for p in range(P):
        pw[p, p // 8] = float(1 << (p % 8))
    return pw


def _select(medians):
    """Live-feature index set and padded width FP."""
    m = np.asarray(medians, dtype=np.float32)
    idx = np.flatnonzero(m > 0)
    fs = int(idx.size)
    fp, _, _, _ = _geom(max(fs, 1))
    return m, idx, fs, fp


def _in_maps(inputs, medians):
    x = np.asarray(inputs, dtype=np.float32)
    m, idx, fs, fp = _select(medians)
    fp, c, r, cq = _geom(fs)
    # Gathered medians, padded with +inf (pad columns compare False).
    m2 = np.full(fp, np.inf, dtype=np.float32)
    m2[:fs] = m[idx]
    med = np.ascontiguousarray(
        np.broadcast_to(
            m2.reshape(1, c, cq), (P // 4 // c, c, cq)
        ).reshape(P // 4, cq)
    )
    pw = _pack_weights()
    xg = x[:, idx]  # [B, fs] gathered live columns
    maps = []
    for ci in range(N_CORES):
        xc = np.zeros((ROWS_PER_CORE, fp), dtype=np.float32)
        xc[:, :fs] = xg[ci * ROWS_PER_CORE : (ci + 1) * ROWS_PER_CORE]
        maps.append({"x": xc, "med": med, "pw": pw})
    return maps


def _decode(packed, fp):
    """[k*G, cq] u8 -> [ROWS_PER_CORE, fp] 0/1 rows for one core."""
    fp, c, r, cq = _geom(fp)
    k = ROWS_PER_CORE // r
    a = packed.reshape(k, G, 1, cq)
    bits = np.unpackbits(a, axis=2, bitorder="little")  # [i, g, kbit, j]
    # partition p = 8g + kbit -> (row p//c, segment p%c)
    bits = bits.reshape(k, P, cq).reshape(k, r, c, cq)
    return bits.reshape(ROWS_PER_CORE, fp)


def kernel(inputs, medians):
    m, idx, fs, fp = _select(medians)
    if fs == 0:
        return np.zeros((np.asarray(inputs).shape[0], m.size), dtype=bool)
    in_maps = _in_maps(inputs, medians)
    last_err = None
    for _ in range(3):  # transient axon/NRT failures happen; retry
        try:
            nc = _build(fs)
            res = run_bass_kernel_spmd(nc, in_maps, list(range(N_CORES))).results
            break
        except Exception as e:  # noqa: BLE001
            last_err = e
    else:
        raise last_err
    gathered = np.concatenate(
        [_decode(r["out"], fs) for r in res], axis=0
    )
    out = np.zeros((gathered.shape[0], m.size), dtype=bool)
    out[:, idx] = gathered[:, :fs].astype(bool)
    return out
